# revision 8
# baseline (speedup 1.0000x reference)
"""Trainium2 Bass kernel for nn_MetapathAggregation (gnn_message_passing).

Strategy (8 NeuronCores, SPMD):
  - Nodes sharded by dst: core c owns nodes [c*3750, (c+1)*3750) of both types,
    padded to 3840 = 30 windows x 128.
  - Edges partitioned by dst core, sorted by dst, padded per 128-node window to
    a tile structure shared across cores (T[w] tiles of 128 edges each).
  - spmm = dma_gather of bf16 feature rows (512B each) + one-hot*val selection
    matrix built on DVE + PE matmul accumulating into PSUM per window.
  - One bf16 AllGather of h_B at the metapath boundary; the independent
    feat_B branch (spmm3 + proj2) overlaps it.
  - Node pipeline (proj/LN/2-token-MHA) in row-major tiles with epilogues
    batched across windows.
"""

import sys
import types

import numpy as np
import ml_dtypes

import concourse.bass as bass
import concourse.bacc as bacc
import concourse.mybir as mybir
import concourse.tile as tile
from concourse.bass_utils import run_bass_kernel_spmd
from concourse.masks import make_identity

F32 = mybir.dt.float32
BF16 = mybir.dt.bfloat16
I16 = mybir.dt.int16
ALU = mybir.AluOpType
ACTF = mybir.ActivationFunctionType
AX = mybir.AxisListType

N = 30000          # nodes per type
V = 2              # views
D = 128            # feature dim
E = 480000         # edges per direction
H = 4              # attention heads
NCORES = 8
NLOC = N // NCORES          # 3750 nodes per core
NWIN = (NLOC + 127) // 128  # 30 windows
NPAD = NWIN * 128           # 3840
GE = 6                      # windows per epilogue group (NWIN % GE == 0)
EPS_LN = 1e-5

_bf = ml_dtypes.bfloat16


def _ensure_profile_hook():
    """Install antenv.axon_hooks shim so trace=True works under axon."""
    if "antenv.axon_hooks" in sys.modules:
        return
    mod = types.ModuleType("antenv.axon_hooks")
    mod._hook = None

    def set_axon_ntff_profile_hook(h):
        mod._hook = h

    def get_axon_ntff_profile_hook():
        return mod._hook

    mod.set_axon_ntff_profile_hook = set_axon_ntff_profile_hook
    mod.get_axon_ntff_profile_hook = get_axon_ntff_profile_hook
    sys.modules["antenv.axon_hooks"] = mod
    try:
        import antenv
        antenv.axon_hooks = mod
        from trn_agent_boot.trn_boot import _ntff_profile_via_ctypes
        hook = _ntff_profile_via_ctypes("/opt/axon/libaxon_pjrt.so")
        if hook is not None:
            mod._hook = hook
    except Exception:
        pass


def _prep_edges(src, dst, val):
    """Partition edges by dst core, sort by dst, pad per window.

    Returns (T, per_core) where T[w] = tiles in window w (shared across cores)
    and per_core[c] = dict(gidx=[128, NT*8] i16, slot=[128, NT] f32,
    val=[128, NT] f32)."""
    order = np.argsort(dst, kind="stable")
    src_s, dst_s, val_s = src[order], dst[order], val[order]
    cores = []
    cnts = np.zeros((NCORES, NWIN), np.int64)
    for c in range(NCORES):
        lo = np.searchsorted(dst_s, c * NLOC)
        hi = np.searchsorted(dst_s, (c + 1) * NLOC)
        dl = dst_s[lo:hi] - c * NLOC
        cores.append((src_s[lo:hi], dl, val_s[lo:hi]))
        cnts[c] = np.bincount(dl >> 7, minlength=NWIN)
    T = np.maximum(1, (cnts.max(axis=0) + 127) // 128)
    tile_base = np.concatenate([[0], np.cumsum(T)])  # [NWIN+1]
    NT = int(tile_base[-1])
    per_core = []
    for c in range(NCORES):
        s, dl, v = cores[c]
        w = dl >> 7
        win_start = np.searchsorted(w, np.arange(NWIN))
        pos_in_win = np.arange(len(dl)) - win_start[w]
        pos = tile_base[w] * 128 + pos_in_win
        gidx_flat = np.zeros(NT * 128, np.int16)
        slot_flat = np.zeros(NT * 128, np.float32)
        val_flat = np.zeros(NT * 128, np.float32)
        gidx_flat[pos] = s.astype(np.int16)
        slot_flat[pos] = (dl & 127).astype(np.float32)
        val_flat[pos] = v
        gidx = np.tile(gidx_flat.reshape(-1, 16).T, (8, 1)).copy()  # [128, NT*8]
        slot = slot_flat.reshape(NT, 128).T.copy()                  # [128, NT]
        valm = val_flat.reshape(NT, 128).T.copy()                   # [128, NT]
        per_core.append({"gidx": gidx, "slot": slot, "val": valm})
    return T, tile_base, NT, per_core


def _bc(ap, n):
    """[P, ...] AP -> [P, n, ...] with stride-0 broadcast dim inserted."""
    return bass.AP(ap.tensor, ap.offset, [ap.ap[0], [0, n], *ap.ap[1:]])


def _build(T_ab, base_ab, NT_ab, T_ba, base_ba, NT_ba):
    nc = bacc.Bacc("TRN2", target_bir_lowering=False)

    # ---- DRAM I/O ----
    fa_d = nc.dram_tensor("fa", [N, V * D], BF16, kind="ExternalInput")
    fb_d = nc.dram_tensor("fb", [N, V * D], BF16, kind="ExternalInput")
    gidx_ab_d = nc.dram_tensor("gidx_ab", [128, NT_ab * 8], I16, kind="ExternalInput")
    slot_ab_d = nc.dram_tensor("slot_ab", [128, NT_ab], F32, kind="ExternalInput")
    val_ab_d = nc.dram_tensor("val_ab", [128, NT_ab], F32, kind="ExternalInput")
    gidx_ba_d = nc.dram_tensor("gidx_ba", [128, NT_ba * 8], I16, kind="ExternalInput")
    slot_ba_d = nc.dram_tensor("slot_ba", [128, NT_ba], F32, kind="ExternalInput")
    val_ba_d = nc.dram_tensor("val_ba", [128, NT_ba], F32, kind="ExternalInput")
    w1t_d = nc.dram_tensor("w1t", [D, D], BF16, kind="ExternalInput")
    w2t_d = nc.dram_tensor("w2t", [D, D], BF16, kind="ExternalInput")
    wint_d = nc.dram_tensor("wint", [D, 3 * D], BF16, kind="ExternalInput")
    woutt_d = nc.dram_tensor("woutt", [D, D], BF16, kind="ExternalInput")
    # replicated per-feature vectors [128, x] f32
    reps_d = {}
    for name, width in [
        ("b1r", D), ("g1r", D), ("be1r", D), ("b2r", D), ("g2r", D), ("be2r", D),
        ("binr", 3 * D), ("boutr", D), ("lghr", D), ("lbhr", D), ("iota", D),
    ]:
        reps_d[name] = nc.dram_tensor(name, [128, width], F32, kind="ExternalInput")
    y_d = nc.dram_tensor("y", [NLOC, V, D], F32, kind="ExternalOutput")

    with tile.TileContext(nc) as tc:
        import contextlib
        ctx = contextlib.ExitStack()
        with ctx:
            consts = ctx.enter_context(tc.tile_pool(name="consts", bufs=1))
            gpool = ctx.enter_context(tc.tile_pool(name="gather", bufs=3))
            ppool = ctx.enter_context(tc.tile_pool(name="pbuild", bufs=3))
            spool = ctx.enter_context(tc.tile_pool(name="scratch", bufs=2))
            epool = ctx.enter_context(tc.tile_pool(name="epi", bufs=1))
            persist = ctx.enter_context(tc.tile_pool(name="persist", bufs=1))
            spsum = ctx.enter_context(tc.tile_pool(name="spmm_ps", bufs=2, space="PSUM"))
            mpsum = ctx.enter_context(tc.tile_pool(name="mm_ps", bufs=3, space="PSUM"))
            dram = ctx.enter_context(tc.tile_pool(name="dram", bufs=1, space="DRAM"))

            # ---- constants into SBUF ----
            cst = {}
            for name in reps_d:
                t = consts.tile([128, reps_d[name].shape[1]], F32, tag=f"c_{name}")
                nc.sync.dma_start(out=t[:], in_=reps_d[name][:, :])
                cst[name] = t
            w1t = consts.tile([D, D], BF16, tag="w1t")
            nc.sync.dma_start(out=w1t[:], in_=w1t_d[:, :])
            w2t = consts.tile([D, D], BF16, tag="w2t")
            nc.sync.dma_start(out=w2t[:], in_=w2t_d[:, :])
            wint = consts.tile([D, 3 * D], BF16, tag="wint")
            nc.sync.dma_start(out=wint[:], in_=wint_d[:, :])
            woutt = consts.tile([D, D], BF16, tag="woutt")
            nc.sync.dma_start(out=woutt[:], in_=woutt_d[:, :])
            ident_b = consts.tile([128, 128], BF16, tag="identb")
            make_identity(nc, ident_b[:])
            ident_f = consts.tile([128, 128], F32, tag="identf")
            make_identity(nc, ident_f[:])
            eps24 = consts.tile([128, 1], F32, tag="eps24")
            nc.vector.memset(eps24[:], 1e-24)
            epsln = consts.tile([128, 1], F32, tag="epsln")
            nc.vector.memset(epsln[:], EPS_LN)
            gidx_ab = consts.tile([128, NT_ab * 8], I16, tag="gidx_ab")
            nc.sync.dma_start(out=gidx_ab[:], in_=gidx_ab_d[:, :])
            slot_ab = consts.tile([128, NT_ab], F32, tag="slot_ab")
            nc.sync.dma_start(out=slot_ab[:], in_=slot_ab_d[:, :])
            val_ab = consts.tile([128, NT_ab], F32, tag="val_ab")
            nc.sync.dma_start(out=val_ab[:], in_=val_ab_d[:, :])
            gidx_ba = consts.tile([128, NT_ba * 8], I16, tag="gidx_ba")
            nc.sync.dma_start(out=gidx_ba[:], in_=gidx_ba_d[:, :])
            slot_ba = consts.tile([128, NT_ba], F32, tag="slot_ba")
            nc.sync.dma_start(out=slot_ba[:], in_=slot_ba_d[:, :])
            val_ba = consts.tile([128, NT_ba], F32, tag="val_ba")
            nc.sync.dma_start(out=val_ba[:], in_=val_ba_d[:, :])
            iota = cst["iota"]

            # persistent h1/h2 [128, NWIN, V, D] bf16
            h1 = persist.tile([128, NWIN, V, D], BF16, tag="h1")
            h2 = persist.tile([128, NWIN, V, D], BF16, tag="h2")

            hb_sh = dram.tile([NPAD, V * D], BF16)      # this core's h_B shard
            hb_full = nc.dram_tensor("hb_full", [N, V * D], BF16,
                                     addr_space="Shared")  # allgathered h_B

            def spmm_window(w, T, base, gidx, slot, val, src_ap, msg_tag):
                """Gather + segment-matmul for window w. Returns PSUM [128, V, D]."""
                Tw = int(T[w])
                b = int(base[w])
                msg = gpool.tile([128, Tw, V * D], BF16, tag=msg_tag)
                nc.gpsimd.dma_gather(
                    out_ap=msg[:],
                    in_ap=src_ap,
                    idxs_ap=gidx[:, b * 8:(b + Tw) * 8],
                    num_idxs=Tw * 128,
                    num_idxs_reg=Tw * 128,
                    elem_size=V * D,
                    single_packet=False,
                )
                acc = spsum.tile([128, V, D], F32, tag="spmm")
                accf = acc[:].rearrange("p v d -> p (v d)")
                for t in range(Tw):
                    tt = b + t
                    P = ppool.tile([128, 128], BF16, tag="P")
                    nc.vector.tensor_scalar(
                        out=P[:], in0=iota[:],
                        scalar1=slot[:, tt:tt + 1], scalar2=val[:, tt:tt + 1],
                        op0=ALU.is_equal, op1=ALU.mult,
                    )
                    nc.tensor.matmul(accf, lhsT=P[:], rhs=msg[:, t, :],
                                     start=(t == 0), stop=(t == Tw - 1))
                return acc

            def l2norm_to(acc, out_bf_ap_by_view):
                """l2norm(acc[128, V, D]) per view -> write bf16 views."""
                sq = spool.tile([128, D], F32, tag="l2sq")
                ss = spool.tile([128, V], F32, tag="l2ss")
                for v in range(V):
                    nc.scalar.activation(out=sq[:], in_=acc[:, v, :],
                                         func=ACTF.Square,
                                         accum_out=ss[:, v:v + 1])
                sn = spool.tile([128, V], F32, tag="l2sn")
                nc.scalar.activation(out=sn[:], in_=ss[:], func=ACTF.Sqrt,
                                     bias=eps24[:, 0:1])
                ri = spool.tile([128, V], F32, tag="l2ri")
                nc.vector.reciprocal(out=ri[:], in_=sn[:])
                for v in range(V):
                    nc.vector.tensor_scalar_mul(out_bf_ap_by_view(v), acc[:, v, :],
                                                ri[:, v:v + 1])

            # ---------- Stage A: spmm_ab -> h_B shard ----------
            for w in range(NWIN):
                acc = spmm_window(w, T_ab, base_ab, gidx_ab, slot_ab, val_ab,
                                  fa_d[:, :], "msg")
                hbw = spool.tile([128, V, D], BF16, tag="hbw")
                l2norm_to(acc, lambda v: hbw[:, v, :])
                nc.sync.dma_start(
                    out=hb_sh[w * 128:(w + 1) * 128, :],
                    in_=hbw[:].rearrange("p v d -> p (v d)"))

            # ---------- Stage B: spmm3 (feat_B) -> h_A2 -> proj2 -> h2 ----------
            # ---------- Stage C: AllGather h_B (emitted after B's gathers) ----
            # ---------- Stage D: spmm2 (h_B full) -> h_A1 -> proj1 -> h1 ------
            def proj_stage(src_ap, msg_tag, wt, brep, grep, berep, hout):
                """Full branch: spmm over ba edges + proj, grouped epilogues."""
                for g in range(NWIN // GE):
                    B = GE * V
                    t = epool.tile([128, B, D], F32, tag="ep_t")
                    for wi in range(GE):
                        w = g * GE + wi
                        acc = spmm_window(w, T_ba, base_ba, gidx_ba, slot_ba,
                                          val_ba, src_ap, msg_tag)
                        ha = spool.tile([128, V, D], BF16, tag="ha")
                        l2norm_to(acc, lambda v: ha[:, v, :])
                        for v in range(V):
                            pt = mpsum.tile([128, 128], BF16, tag="mmt")
                            nc.tensor.transpose(out=pt[:], in_=ha[:, v, :],
                                                identity=ident_b[:])
                            xT = spool.tile([128, 128], BF16, tag="xT")
                            nc.scalar.copy(out=xT[:], in_=pt[:])
                            zz = mpsum.tile([128, D], F32, tag="mm")
                            nc.tensor.matmul(zz[:], lhsT=xT[:], rhs=wt[:],
                                             start=True, stop=True)
                            nc.vector.tensor_tensor(out=t[:, wi * V + v, :],
                                                    in0=zz[:], in1=brep[:],
                                                    op=ALU.add)
                    # batched LN + relu epilogue over [128, GE*V, D]
                    mu = epool.tile([128, B], F32, tag="ep_mu")
                    nc.vector.tensor_reduce(out=mu[:], in_=t[:], axis=AX.X,
                                            op=ALU.add)
                    nc.vector.tensor_scalar_mul(mu[:], mu[:], 1.0 / D)
                    c = t
                    nc.vector.tensor_tensor(out=c[:], in0=t[:],
                                            in1=mu[:].to_broadcast([128, B, D]),
                                            op=ALU.subtract)
                    sq = epool.tile([128, B, D], F32, tag="e_tmp", bufs=2)
                    nc.vector.tensor_tensor(out=sq[:], in0=c[:], in1=c[:],
                                            op=ALU.mult)
                    var = epool.tile([128, B], F32, tag="ep_var")
                    nc.vector.tensor_reduce(out=var[:], in_=sq[:], axis=AX.X,
                                            op=ALU.add)
                    rs = epool.tile([128, B], F32, tag="ep_rs")
                    nc.scalar.activation(out=rs[:], in_=var[:], func=ACTF.Sqrt,
                                         scale=1.0 / D, bias=epsln[:, 0:1])
                    nc.vector.reciprocal(out=rs[:], in_=rs[:])
                    nc.vector.tensor_tensor(out=c[:], in0=c[:],
                                            in1=rs[:].to_broadcast([128, B, D]),
                                            op=ALU.mult)
                    nc.vector.tensor_tensor(out=c[:], in0=c[:], in1=_bc(grep[:], B),
                                            op=ALU.mult)
                    nc.vector.tensor_tensor(out=c[:], in0=c[:], in1=_bc(berep[:], B),
                                            op=ALU.add)
                    # relu -> bf16 h (ACT)
                    hv = hout[:, g * GE:(g + 1) * GE, :, :].rearrange(
                        "p w v d -> p (w v) d")
                    nc.scalar.activation(out=hv, in_=c[:], func=ACTF.Relu)

            proj_stage(fb_d[:, :], "msg", w2t, cst["b2r"], cst["g2r"],
                       cst["be2r"], h2)

            nc.gpsimd.collective_compute(
                "AllGather", ALU.bypass,
                replica_groups=[list(range(NCORES))],
                ins=[hb_sh[0:NLOC, :].opt()],
                outs=[hb_full.ap().opt()],
            )

            proj_stage(hb_full.ap(), "msg", w1t, cst["b1r"], cst["g1r"],
                       cst["be1r"], h1)

            # ---------- Stage E: MHA over P=2 + LN + mean ----------
            binr, boutr, lghr, lbhr = (cst["binr"], cst["boutr"], cst["lghr"],
                                       cst["lbhr"])
            for g in range(NWIN // GE):
                B = GE * V
                qkv = [None, None]
                for p, hsrc in enumerate((h1, h2)):
                    qk = epool.tile([128, B, 3 * D], BF16, tag=f"qkv{p}")
                    for wi in range(GE):
                        w = g * GE + wi
                        for v in range(V):
                            pt = mpsum.tile([128, 128], BF16, tag="mmt")
                            nc.tensor.transpose(out=pt[:], in_=hsrc[:, w, v, :],
                                                identity=ident_b[:])
                            xT = spool.tile([128, 128], BF16, tag="xT")
                            nc.scalar.copy(out=xT[:], in_=pt[:])
                            qp = mpsum.tile([128, 3 * D], F32, tag="mm")
                            nc.tensor.matmul(qp[:], lhsT=xT[:], rhs=wint[:],
                                             start=True, stop=True)
                            nc.vector.tensor_tensor(out=qk[:, wi * V + v, :],
                                                    in0=qp[:], in1=binr[:],
                                                    op=ALU.add)
                    qkv[p] = qk
                # scores s_pq = sum_e q_p * k_q per head  [128, B*H]
                scr = epool.tile([128, B, D], F32, tag="e_tmp", bufs=2)
                s = {}
                for p in range(2):
                    for q in range(2):
                        qv = qkv[p][:, :, 0:D]
                        kv = qkv[q][:, :, D:2 * D]
                        st = epool.tile([128, B * H], F32, tag=f"e_s{p}{q}")
                        nc.vector.tensor_tensor(out=scr[:], in0=qv, in1=kv,
                                                op=ALU.mult)
                        nc.vector.tensor_reduce(
                            out=st[:],
                            in_=scr[:].rearrange("p b (h e) -> p b h e", h=H),
                            axis=AX.X, op=ALU.add)
                        s[(p, q)] = st
                att = [None, None]
                for p in range(2):
                    dlog = epool.tile([128, B * H], F32, tag=f"e_d{p}")
                    nc.vector.tensor_tensor(out=dlog[:], in0=s[(p, 0)][:],
                                            in1=s[(p, 1)][:], op=ALU.subtract)
                    a1 = epool.tile([128, B * H], F32, tag=f"e_a1{p}")
                    nc.scalar.activation(out=a1[:], in_=dlog[:], func=ACTF.Sigmoid)
                    a2 = epool.tile([128, B * H], F32, tag=f"e_a2{p}")
                    nc.vector.tensor_scalar(out=a2[:], in0=a1[:], scalar1=-1.0,
                                            scalar2=1.0, op0=ALU.mult, op1=ALU.add)
                    # o_p = a1*v1 + a2*v2
                    o = epool.tile([128, B, D], F32, tag=f"e_o{p}")
                    v1 = qkv[0][:, :, 2 * D:3 * D]
                    v2 = qkv[1][:, :, 2 * D:3 * D]
                    def abc(a):  # [128, B*H] -> [p, b, h, 0x32] broadcast AP
                        aa = a[:].rearrange("p (b h) -> p b h", h=H)
                        return bass.AP(aa.tensor, aa.offset,
                                       [*aa.ap, [0, D // H]])
                    nc.vector.tensor_tensor(
                        out=o[:].rearrange("p b (h e) -> p b h e", h=H),
                        in0=v1.rearrange("p b (h e) -> p b h e", h=H),
                        in1=abc(a1), op=ALU.mult)
                    scr2 = epool.tile([128, B, D], F32, tag="e_tmp", bufs=2)
                    nc.vector.tensor_tensor(
                        out=scr2[:].rearrange("p b (h e) -> p b h e", h=H),
                        in0=v2.rearrange("p b (h e) -> p b h e", h=H),
                        in1=abc(a2), op=ALU.mult)
                    nc.vector.tensor_tensor(out=o[:], in0=o[:], in1=scr2[:],
                                            op=ALU.add)
                    # attn_out = o @ WoutT + bout ; residual += x_p
                    ao = epool.tile([128, B, D], F32, tag=f"e_ao{p}")
                    for bi in range(B):
                        pt = mpsum.tile([128, 128], F32, tag="mm")
                        nc.tensor.transpose(out=pt[:], in_=o[:, bi, :],
                                            identity=ident_f[:])
                        oT = spool.tile([128, 128], BF16, tag="xT")
                        nc.scalar.copy(out=oT[:], in_=pt[:])
                        ap = mpsum.tile([128, D], F32, tag="mm")
                        nc.tensor.matmul(ap[:], lhsT=oT[:], rhs=woutt[:],
                                         start=True, stop=True)
                        nc.vector.tensor_tensor(out=ao[:, bi, :], in0=ap[:],
                                                in1=boutr[:], op=ALU.add)
                    hsrc = (h1, h2)[p]
                    xv = hsrc[:, g * GE:(g + 1) * GE, :, :].rearrange(
                        "p w v d -> p (w v) d")
                    nc.vector.tensor_tensor(out=ao[:], in0=ao[:], in1=xv,
                                            op=ALU.add)
                    # LN with lng/2, lnb/2
                    mu = epool.tile([128, B], F32, tag=f"e_mu{p}")
                    nc.vector.tensor_reduce(out=mu[:], in_=ao[:], axis=AX.X,
                                            op=ALU.add)
                    nc.vector.tensor_scalar_mul(mu[:], mu[:], 1.0 / D)
                    nc.vector.tensor_tensor(out=ao[:], in0=ao[:],
                                            in1=mu[:].to_broadcast([128, B, D]),
                                            op=ALU.subtract)
                    sq = epool.tile([128, B, D], F32, tag="e_tmp", bufs=2)
                    nc.vector.tensor_tensor(out=sq[:], in0=ao[:], in1=ao[:],
                                            op=ALU.mult)
                    var = epool.tile([128, B], F32, tag=f"e_var{p}")
                    nc.vector.tensor_reduce(out=var[:], in_=sq[:], axis=AX.X,
                                            op=ALU.add)
                    rs = epool.tile([128, B], F32, tag=f"e_rs{p}")
                    nc.scalar.activation(out=rs[:], in_=var[:], func=ACTF.Sqrt,
                                         scale=1.0 / D, bias=epsln[:, 0:1])
                    nc.vector.reciprocal(out=rs[:], in_=rs[:])
                    nc.vector.tensor_tensor(out=ao[:], in0=ao[:],
                                            in1=rs[:].to_broadcast([128, B, D]),
                                            op=ALU.mult)
                    nc.vector.tensor_tensor(out=ao[:], in0=ao[:], in1=_bc(lghr[:], B),
                                            op=ALU.mult)
                    nc.vector.tensor_tensor(out=ao[:], in0=ao[:], in1=_bc(lbhr[:], B),
                                            op=ALU.add)
                    att[p] = ao
                yg = epool.tile([128, GE, V, D], F32, tag="e_y")
                nc.vector.tensor_tensor(
                    out=yg[:].rearrange("p w v d -> p (w v) d"),
                    in0=att[0][:], in1=att[1][:], op=ALU.add)
                for wi in range(GE):
                    w = g * GE + wi
                    lo = w * 128
                    rows = min(128, NLOC - lo)
                    if rows <= 0:
                        continue
                    nc.sync.dma_start(out=y_d[lo:lo + rows, :, :],
                                      in_=yg[:rows, wi, :, :])
    nc.finalize()
    return nc


def _enable_jax_cache():
    try:
        import jax
        jax.config.update("jax_compilation_cache_dir", "/tmp/jax_kernel_cache")
        jax.config.update("jax_persistent_cache_min_entry_size_bytes", -1)
        jax.config.update("jax_persistent_cache_min_compile_time_secs", 0.0)
    except Exception:
        pass


def kernel(feat_A, feat_B, src_ab, dst_ab, val_ab, src_ba, dst_ba, val_ba,
           W1, b1, g1, be1, W2, b2, g2, be2, Win, bin_, Wout, bout, lng, lnb):
    _ensure_profile_hook()
    _enable_jax_cache()
    feat_A = np.asarray(feat_A, np.float32)
    feat_B = np.asarray(feat_B, np.float32)
    src_ab = np.asarray(src_ab, np.int32)
    dst_ab = np.asarray(dst_ab, np.int32)
    val_ab = np.asarray(val_ab, np.float32)
    src_ba = np.asarray(src_ba, np.int32)
    dst_ba = np.asarray(dst_ba, np.int32)
    val_ba = np.asarray(val_ba, np.float32)
    W1 = np.asarray(W1, np.float32)
    W2 = np.asarray(W2, np.float32)
    Win = np.asarray(Win, np.float32)
    Wout = np.asarray(Wout, np.float32)
    b1 = np.asarray(b1, np.float32)
    g1 = np.asarray(g1, np.float32)
    be1 = np.asarray(be1, np.float32)
    b2 = np.asarray(b2, np.float32)
    g2 = np.asarray(g2, np.float32)
    be2 = np.asarray(be2, np.float32)
    bin_ = np.asarray(bin_, np.float32)
    bout = np.asarray(bout, np.float32)
    lng = np.asarray(lng, np.float32)
    lnb = np.asarray(lnb, np.float32)

    T_ab, base_ab, NT_ab, pc_ab = _prep_edges(src_ab, dst_ab, val_ab)
    T_ba, base_ba, NT_ba, pc_ba = _prep_edges(src_ba, dst_ba, val_ba)

    fa = feat_A.reshape(N, V * D).astype(_bf)
    fb = feat_B.reshape(N, V * D).astype(_bf)
    rep = lambda x: np.tile(x[None, :], (128, 1)).astype(np.float32)
    wint = Win.T.copy()
    binp = bin_.copy()
    sc = 1.0 / np.sqrt(D // H)
    wint[:, :D] *= sc
    binp[:D] *= sc
    common = {
        "fa": fa, "fb": fb,
        "w1t": W1.T.astype(_bf).copy(), "w2t": W2.T.astype(_bf).copy(),
        "wint": wint.astype(_bf), "woutt": Wout.T.astype(_bf).copy(),
        "b1r": rep(b1), "g1r": rep(g1), "be1r": rep(be1),
        "b2r": rep(b2), "g2r": rep(g2), "be2r": rep(be2),
        "binr": rep(binp), "boutr": rep(bout),
        "lghr": rep(lng * 0.5), "lbhr": rep(lnb * 0.5),
        "iota": np.tile(np.arange(D, dtype=np.float32), (128, 1)),
    }
    in_maps = []
    for c in range(NCORES):
        m = dict(common)
        m["gidx_ab"] = pc_ab[c]["gidx"]
        m["slot_ab"] = pc_ab[c]["slot"]
        m["val_ab"] = pc_ab[c]["val"]
        m["gidx_ba"] = pc_ba[c]["gidx"]
        m["slot_ba"] = pc_ba[c]["slot"]
        m["val_ba"] = pc_ba[c]["val"]
        in_maps.append(m)

    nc = _build(T_ab, base_ab, NT_ab, T_ba, base_ba, NT_ba)
    trace = bool(int(__import__("os").environ.get("KERNEL_TRACE", "0")))
    res = run_bass_kernel_spmd(nc, in_maps, core_ids=list(range(NCORES)),
                               trace=trace)
    kernel.last_result = res
    y = np.concatenate([res.results[c]["y"] for c in range(NCORES)], axis=0)
    return y
